# revision 6
# baseline (speedup 1.0000x reference)
"""Single-head causal attention kernel for TRN2 (8 NeuronCores, data-parallel).

Problem: x[256,256,384] f32, Wq/Wk/Wv[384,64] f32 ->
  out = softmax(mask((x@Wq)(x@Wk)^T/8)) @ (x@Wv)  [256,256,64] f32

Sharding: batch 256 -> 8 cores x 32 batches. Weights replicated.

v2 dataflow per batch (bf16 matmuls, fp32 PSUM):
  S1: PE transposes x -> xT (6x 128x128), DVE copies xT to SBUF (2x_1p)
  S2: PE qkT = [Wq|Wk]-stationary MM streaming xT (3 MMs, full array)
      -> PSUM [128,2,193]-tile also collects v via 6 xt-stationary MMs;
      ACT copies qT (aligned) + v, DVE copies kT (partition shift 64->0)
  S3: PE scores with kT stationary / qT moving -> wei [diag0|diag1|offdiag];
      ACT exp(scale*wei) -> mexp bf16; Pool one in-place affine_select masks
      both diag blocks
  S4: PE att@v (3 MMs, 65 cols: v plus a persistent ones column for the
      softmax denominator); per PAIR of batches: DVE reciprocal + broadcast
      normalize into the store tile; HWDGE stores groups of 4

PE warmup: dummy transposes fill the initial x-load latency so the tensor
engine p-state is ramped when real work arrives.
"""

import numpy as np

B, T, C, H = 256, 256, 384, 64
NCORES = 8
BPC = B // NCORES  # 32 batches per core
CCH = C // 128  # 3 contraction chunks
TCH = T // 128  # 2 t-chunks

LOADG = [2, 2, 4, 8, 8, 8]  # batches per x-load DMA
STOREG = [4] * 7 + [2, 2]  # batches per out-store DMA (pair-aligned)
NWARM = 26  # dummy PE transposes to ramp the p-state during load latency

_CACHE = {}


def _build():
    import concourse.bass as bass
    import concourse.mybir as mybir
    import concourse.tile as tile
    from concourse import bacc
    from concourse.bass import broadcast_tensor_aps
    from concourse.masks import make_identity

    fp32 = mybir.dt.float32
    bf16 = mybir.dt.bfloat16

    nc = bacc.Bacc()
    x_d = nc.declare_dram_parameter("x", [BPC, T, C], fp32, isOutput=False)
    wq_d = nc.declare_dram_parameter("wq", [C, H], fp32, isOutput=False)
    wk_d = nc.declare_dram_parameter("wk", [C, H], fp32, isOutput=False)
    wv_d = nc.declare_dram_parameter("wv", [C, H], fp32, isOutput=False)
    out_d = nc.declare_dram_parameter("out", [BPC, T, H], fp32, isOutput=True)

    with tile.TileContext(nc) as tc:
        with (
            tc.tile_pool(name="singles", bufs=1) as singles,
            tc.tile_pool(name="xin", bufs=2) as xin,
            tc.tile_pool(name="xtp", bufs=2) as xtp,
            tc.tile_pool(name="qkp", bufs=2) as qkp,
            tc.tile_pool(name="mxp", bufs=2) as mxp,
            tc.tile_pool(name="recp", bufs=2) as recp,
            tc.tile_pool(name="outp", bufs=2) as outp,
            tc.tile_pool(name="ps_xt", bufs=2, space="PSUM") as ps_xt,
            tc.tile_pool(name="ps_qv", bufs=2, space="PSUM") as ps_qv,
            tc.tile_pool(name="ps_wei", bufs=2, space="PSUM") as ps_wei,
            tc.tile_pool(name="ps_oa", bufs=2, space="PSUM") as ps_oa,
        ):
            # issue the first x loads before anything else queues on gpsimd
            xbs = []  # (tile, start_batch, nbatch)
            starts = np.cumsum([0] + LOADG)

            def issue_load(gi):
                n = LOADG[gi]
                s = int(starts[gi])
                xb = xin.tile([128, n, TCH, C], bf16, name=f"xb_{gi}",
                              tag=f"xb{n}")
                nc.gpsimd.dma_start(
                    out=xb,
                    in_=x_d[s:s + n].rearrange("n (c p) f -> p n c f", p=128),
                )
                xbs.append((xb, s, n))

            for gi in range(3):
                issue_load(gi)

            # --- constants ---
            ident = singles.tile([128, 128], bf16)
            make_identity(nc, ident)

            # W staging: HWDGE fp32 loads, cast to bf16 on ACT
            # wstage cols 0:64=Wq, 64:128=Wk, 128:192=Wv
            wstage = singles.tile([128, CCH, 3 * H], fp32, tag="wstage")
            for wi, wd in enumerate((wq_d, wk_d, wv_d)):
                nc.sync.dma_start(
                    out=wstage[:, :, wi * H:(wi + 1) * H],
                    in_=wd.rearrange("(c p) h -> p c h", p=128),
                )
            wqk_sb = singles.tile([128, CCH, 2 * H], bf16)
            nc.scalar.copy(wqk_sb, wstage[:, :, 0:2 * H])
            wv_sb = singles.tile([128, CCH, H], bf16)
            nc.scalar.copy(wv_sb, wstage[:, :, 2 * H:3 * H])

            # persistent v tiles with a ones column at [:, :, H]
            vsbs = []
            for vi in range(3):
                vt = singles.tile([128, TCH, H + 1], bf16, name=f"vsb{vi}")
                nc.gpsimd.memset(vt[:, :, H:H + 1], 1.0)
                vsbs.append(vt)

            # PE warmup: dummy transposes while the first x load lands
            for wu in range(NWARM):
                wps = ps_xt.tile([128, CCH, 256], bf16, tag="xt")
                nc.tensor.transpose(wps[:, 0, 0:128], ident, ident)

            # --- pipeline state ---
            state = {}  # b -> dict
            ld = {"next": 3, "cur": 0}

            sg_iter = iter(STOREG)
            sg = {"n": 0, "left": 0, "start": 0, "osb": None}

            def stage1(b):
                if b >= xbs[ld["cur"]][1] + xbs[ld["cur"]][2]:
                    ld["cur"] += 1
                    if ld["next"] < len(LOADG):
                        issue_load(ld["next"])
                        ld["next"] += 1
                xb, xs, xn = xbs[ld["cur"]]
                bi = b - xs

                xt_ps = ps_xt.tile([128, CCH, 256], bf16, tag="xt")
                for c in range(CCH):
                    for t in range(TCH):
                        nc.tensor.transpose(
                            xt_ps[:, c, t * 128:(t + 1) * 128],
                            xb[:, bi, t, c * 128:(c + 1) * 128],
                            ident,
                        )
                xt = xtp.tile([128, CCH, 256], bf16, tag="xt_sb")
                nc.vector.tensor_copy(xt, xt_ps)
                state[b] = {"xt": xt}

            def stage2(b):
                st = state[b]
                xt = st["xt"]
                # qv_ps cols 0:256 = qkT (q rows 0:64, k rows 64:128),
                # cols 256:384 = v as (t h)
                qv_ps = ps_qv.tile([128, 386], fp32)
                for c in range(CCH):
                    nc.tensor.matmul(
                        qv_ps[:, 0:256],
                        lhsT=wqk_sb[:, c, :],
                        rhs=xt[:, c, :],
                        start=(c == 0), stop=(c == CCH - 1),
                    )
                for t in range(TCH):
                    for c in range(CCH):
                        nc.tensor.matmul(
                            qv_ps[:, 256 + t * H:256 + (t + 1) * H],
                            lhsT=xt[:, c, t * 128:(t + 1) * 128],
                            rhs=wv_sb[:, c, :],
                            start=(c == 0), stop=(c == CCH - 1),
                        )
                # qkt[64, qk, 256]: q aligned (ACT), k shifted 64->0 (DVE)
                qkt = qkp.tile([64, 2, 256], bf16, tag="qkt")
                nc.scalar.copy(qkt[:, 0, :], qv_ps[0:64, 0:256])
                nc.vector.tensor_copy(qkt[:, 1, :], qv_ps[64:128, 0:256])
                vsb = vsbs[b % 3]
                nc.scalar.copy(
                    vsb[:, :, 0:H],
                    qv_ps[:, 256:384].rearrange("p (t h) -> p t h", t=TCH))
                st["qkt"] = qkt
                st["vsb"] = vsb

            def stage3(b):
                st = state[b]
                qkt = st["qkt"]
                # wei layout [128, 3, 128]: blk0 = s0 x t0 (diag), blk1 =
                # s1 x t1 (diag), blk2 = s0 x t1 (full)
                wei_ps = ps_wei.tile([128, 3, 128], fp32)
                nc.tensor.matmul(
                    wei_ps[:, 0, :],
                    lhsT=qkt[:, 1, 0:128], rhs=qkt[:, 0, 0:128],
                    start=True, stop=True,
                )
                nc.tensor.matmul(
                    wei_ps[:, 2, :],
                    lhsT=qkt[:, 1, 0:128], rhs=qkt[:, 0, 128:256],
                    start=True, stop=True,
                )
                nc.tensor.matmul(
                    wei_ps[:, 1, :],
                    lhsT=qkt[:, 1, 128:256], rhs=qkt[:, 0, 128:256],
                    start=True, stop=True,
                )
                mexp = mxp.tile([128, 3, 128], bf16, tag="mexp")
                nc.scalar.activation(
                    out=mexp, in_=wei_ps,
                    func=mybir.ActivationFunctionType.Exp,
                    scale=float(H) ** -0.5,
                )
                # causal mask on both diag blocks in one op: keep j >= p
                nc.gpsimd.affine_select(
                    out=mexp[:, 0:2, :], in_=mexp[:, 0:2, :],
                    compare_op=mybir.AluOpType.is_ge,
                    fill=0.0, base=0,
                    pattern=[[0, 2], [1, 128]],
                    channel_multiplier=-1,
                )
                st["mexp"] = mexp

            def stage4(b):
                st = state.pop(b)
                mexp, vsb = st["mexp"], st["vsb"]
                pj = b & 1
                if pj == 0:
                    _PAIR["oa"] = ps_oa.tile([128, 2, TCH, H + 1], fp32,
                                             name=f"oa_{b}", tag="oa")
                oa = _PAIR["oa"]
                nc.tensor.matmul(
                    oa[:, pj, 0, :], lhsT=mexp[:, 0, :], rhs=vsb[:, 0, :],
                    start=True, stop=True,
                )
                nc.tensor.matmul(
                    oa[:, pj, 1, :], lhsT=mexp[:, 2, :], rhs=vsb[:, 0, :],
                    start=True, stop=False,
                )
                nc.tensor.matmul(
                    oa[:, pj, 1, :], lhsT=mexp[:, 1, :], rhs=vsb[:, 1, :],
                    start=False, stop=True,
                )

                if sg["left"] == 0:
                    sg["n"] = next(sg_iter)
                    sg["left"] = sg["n"]
                    sg["start"] = b
                    sg["osb"] = outp.tile([128, sg["n"], TCH, H], fp32,
                                          name=f"osb_{b}",
                                          tag=f"osb{sg['n']}")
                osb = sg["osb"]

                if pj == 1:
                    rec = recp.tile([128, 2, TCH], fp32, tag="rec")
                    nc.vector.reciprocal(rec, oa[:, :, :, H])
                    j = b - 1 - sg["start"]
                    o_ap, r_ap = broadcast_tensor_aps(
                        oa[:, :, :, 0:H], rec[:, :, :, None])
                    nc.vector.tensor_mul(osb[:, j:j + 2], o_ap, r_ap)

                sg["left"] -= 1
                if sg["left"] == 0:
                    n = sg["n"]
                    nc.sync.dma_start(
                        out=out_d[sg["start"]:sg["start"] + n].rearrange(
                            "n (c p) h -> p n c h", p=128),
                        in_=osb,
                    )

            _PAIR = {}
            for i in range(BPC + 3):
                if i < BPC:
                    stage1(i)
                if 1 <= i < BPC + 1:
                    stage2(i - 1)
                if 2 <= i < BPC + 2:
                    stage3(i - 2)
                if 3 <= i:
                    stage4(i - 3)
    nc.compile()
    return nc


def _get_nc():
    if "nc" not in _CACHE:
        _CACHE["nc"] = _build()
    return _CACHE["nc"]


def kernel(x, Wq, Wk, Wv):
    from concourse.bass_utils import run_bass_kernel_spmd

    x = np.ascontiguousarray(np.asarray(x, dtype=np.float32))
    Wq = np.ascontiguousarray(np.asarray(Wq, dtype=np.float32))
    Wk = np.ascontiguousarray(np.asarray(Wk, dtype=np.float32))
    Wv = np.ascontiguousarray(np.asarray(Wv, dtype=np.float32))

    nc = _get_nc()
    in_maps = [
        {"x": x[i * BPC:(i + 1) * BPC], "wq": Wq, "wk": Wk, "wv": Wv}
        for i in range(NCORES)
    ]
    res = run_bass_kernel_spmd(nc, in_maps, list(range(NCORES)))
    return np.concatenate([res.results[i]["out"] for i in range(NCORES)], axis=0)


# revision 7
# speedup vs baseline: 1.0323x; 1.0323x over previous
"""Single-head causal attention kernel for TRN2 (8 NeuronCores, data-parallel).

Problem: x[256,256,384] f32, Wq/Wk/Wv[384,64] f32 ->
  out = softmax(mask((x@Wq)(x@Wk)^T/8)) @ (x@Wv)  [256,256,64] f32

Sharding: batch 256 -> 8 cores x 32 batches. Weights replicated.

v2 dataflow per batch (bf16 matmuls, fp32 PSUM):
  S1: PE transposes x -> xT (6x 128x128), DVE copies xT to SBUF (2x_1p)
  S2: PE qkT = [Wq|Wk]-stationary MM streaming xT (3 MMs, full array)
      -> PSUM [128,2,193]-tile also collects v via 6 xt-stationary MMs;
      ACT copies qT (aligned) + v, DVE copies kT (partition shift 64->0)
  S3: PE scores with kT stationary / qT moving -> wei [diag0|diag1|offdiag];
      ACT exp(scale*wei) -> mexp bf16; Pool one in-place affine_select masks
      both diag blocks
  S4: PE att@v (3 MMs, 65 cols: v plus a persistent ones column for the
      softmax denominator); per PAIR of batches: DVE reciprocal + broadcast
      normalize into the store tile; HWDGE stores groups of 4

PE warmup: dummy transposes fill the initial x-load latency so the tensor
engine p-state is ramped when real work arrives.
"""

import numpy as np

B, T, C, H = 256, 256, 384, 64
NCORES = 8
BPC = B // NCORES  # 32 batches per core
CCH = C // 128  # 3 contraction chunks
TCH = T // 128  # 2 t-chunks

LOADG = [2, 2, 4, 8, 8, 8]  # batches per x-load DMA
STOREG = [4] * 7 + [2, 2]  # batches per out-store DMA (pair-aligned)
NWARM = 20  # dummy PE transposes to ramp the p-state during load latency

_CACHE = {}


def _build():
    import concourse.bass as bass
    import concourse.mybir as mybir
    import concourse.tile as tile
    from concourse import bacc
    from concourse.bass import broadcast_tensor_aps
    from concourse.masks import make_identity

    fp32 = mybir.dt.float32
    bf16 = mybir.dt.bfloat16

    nc = bacc.Bacc()
    x_d = nc.declare_dram_parameter("x", [BPC, T, C], fp32, isOutput=False)
    wq_d = nc.declare_dram_parameter("wq", [C, H], fp32, isOutput=False)
    wk_d = nc.declare_dram_parameter("wk", [C, H], fp32, isOutput=False)
    wv_d = nc.declare_dram_parameter("wv", [C, H], fp32, isOutput=False)
    out_d = nc.declare_dram_parameter("out", [BPC, T, H], fp32, isOutput=True)

    with tile.TileContext(nc) as tc:
        with (
            tc.tile_pool(name="singles", bufs=1) as singles,
            tc.tile_pool(name="xin", bufs=2) as xin,
            tc.tile_pool(name="xtp", bufs=2) as xtp,
            tc.tile_pool(name="qkp", bufs=2) as qkp,
            tc.tile_pool(name="mxp", bufs=2) as mxp,
            tc.tile_pool(name="recp", bufs=2) as recp,
            tc.tile_pool(name="outp", bufs=2) as outp,
            tc.tile_pool(name="ps_xt", bufs=2, space="PSUM") as ps_xt,
            tc.tile_pool(name="ps_qv", bufs=2, space="PSUM") as ps_qv,
            tc.tile_pool(name="ps_wei", bufs=2, space="PSUM") as ps_wei,
            tc.tile_pool(name="ps_oa", bufs=2, space="PSUM") as ps_oa,
        ):
            xbs = []  # (tile, start_batch, nbatch)
            starts = np.cumsum([0] + LOADG)

            def issue_load(gi):
                n = LOADG[gi]
                s = int(starts[gi])
                xb = xin.tile([128, n, TCH, C], bf16, name=f"xb_{gi}",
                              tag=f"xb{n}", bufs=(3 if n == 8 else 2))
                nc.gpsimd.dma_start(
                    out=xb,
                    in_=x_d[s:s + n].rearrange("n (c p) f -> p n c f", p=128),
                )
                xbs.append((xb, s, n))

            # setup order matters: identity first (tiny Pool ops), then the
            # first x-load trigger, then PE warmup can start immediately
            # while W loads go out on the Sync HWDGE queue in parallel.
            ident = singles.tile([128, 128], bf16)
            make_identity(nc, ident)
            issue_load(0)

            # W staging: HWDGE fp32 loads, cast to bf16 on ACT
            # wstage cols 0:64=Wq, 64:128=Wk, 128:192=Wv
            wstage = singles.tile([128, CCH, 3 * H], fp32, tag="wstage")
            for wi, wd in enumerate((wq_d, wk_d, wv_d)):
                nc.sync.dma_start(
                    out=wstage[:, :, wi * H:(wi + 1) * H],
                    in_=wd.rearrange("(c p) h -> p c h", p=128),
                )

            # PE warmup: dummy transposes ramp the p-state during load waits
            for wu in range(NWARM):
                wps = ps_xt.tile([128, CCH, 256], bf16, tag="xt")
                nc.tensor.transpose(wps[:, 0, 0:128], ident, ident)

            issue_load(1)
            issue_load(2)

            wqk_sb = singles.tile([128, CCH, 2 * H], bf16)
            nc.scalar.copy(wqk_sb, wstage[:, :, 0:2 * H])
            wv_sb = singles.tile([128, CCH, H], bf16)
            nc.scalar.copy(wv_sb, wstage[:, :, 2 * H:3 * H])

            # persistent v tiles with a ones column at [:, :, H]
            vsbs = []
            for vi in range(3):
                vt = singles.tile([128, TCH, H + 1], bf16, name=f"vsb{vi}")
                nc.gpsimd.memset(vt[:, :, H:H + 1], 1.0)
                vsbs.append(vt)

            # --- pipeline state ---
            state = {}  # b -> dict
            ld = {"next": 3, "cur": 0}

            sg_iter = iter(STOREG)
            sg = {"n": 0, "left": 0, "start": 0, "osb": None}

            def stage1(b):
                if b >= xbs[ld["cur"]][1] + xbs[ld["cur"]][2]:
                    ld["cur"] += 1
                    if ld["next"] < len(LOADG):
                        issue_load(ld["next"])
                        ld["next"] += 1
                xb, xs, xn = xbs[ld["cur"]]
                bi = b - xs

                xt_ps = ps_xt.tile([128, CCH, 256], bf16, tag="xt")
                for c in range(CCH):
                    for t in range(TCH):
                        nc.tensor.transpose(
                            xt_ps[:, c, t * 128:(t + 1) * 128],
                            xb[:, bi, t, c * 128:(c + 1) * 128],
                            ident,
                        )
                xt = xtp.tile([128, CCH, 256], bf16, tag="xt_sb")
                nc.vector.tensor_copy(xt, xt_ps)
                state[b] = {"xt": xt}

            def stage2(b):
                st = state[b]
                xt = st["xt"]
                # qv_ps cols 0:256 = qkT (q rows 0:64, k rows 64:128),
                # cols 256:384 = v as (t h)
                qv_ps = ps_qv.tile([128, 386], fp32)
                for c in range(CCH):
                    nc.tensor.matmul(
                        qv_ps[:, 0:256],
                        lhsT=wqk_sb[:, c, :],
                        rhs=xt[:, c, :],
                        start=(c == 0), stop=(c == CCH - 1),
                    )
                for t in range(TCH):
                    for c in range(CCH):
                        nc.tensor.matmul(
                            qv_ps[:, 256 + t * H:256 + (t + 1) * H],
                            lhsT=xt[:, c, t * 128:(t + 1) * 128],
                            rhs=wv_sb[:, c, :],
                            start=(c == 0), stop=(c == CCH - 1),
                        )
                # qkt[64, qk, 256]: q aligned (ACT), k shifted 64->0 (DVE)
                qkt = qkp.tile([64, 2, 256], bf16, tag="qkt")
                nc.scalar.copy(qkt[:, 0, :], qv_ps[0:64, 0:256])
                nc.vector.tensor_copy(qkt[:, 1, :], qv_ps[64:128, 0:256])
                vsb = vsbs[b % 3]
                nc.scalar.copy(
                    vsb[:, :, 0:H],
                    qv_ps[:, 256:384].rearrange("p (t h) -> p t h", t=TCH))
                st["qkt"] = qkt
                st["vsb"] = vsb

            def stage3(b):
                st = state[b]
                qkt = st["qkt"]
                # wei layout [128, 3, 128]: blk0 = s0 x t0 (diag), blk1 =
                # s1 x t1 (diag), blk2 = s0 x t1 (full)
                wei_ps = ps_wei.tile([128, 3, 128], fp32)
                nc.tensor.matmul(
                    wei_ps[:, 0, :],
                    lhsT=qkt[:, 1, 0:128], rhs=qkt[:, 0, 0:128],
                    start=True, stop=True,
                )
                nc.tensor.matmul(
                    wei_ps[:, 2, :],
                    lhsT=qkt[:, 1, 0:128], rhs=qkt[:, 0, 128:256],
                    start=True, stop=True,
                )
                nc.tensor.matmul(
                    wei_ps[:, 1, :],
                    lhsT=qkt[:, 1, 128:256], rhs=qkt[:, 0, 128:256],
                    start=True, stop=True,
                )
                mexp = mxp.tile([128, 3, 128], bf16, tag="mexp")
                nc.scalar.activation(
                    out=mexp, in_=wei_ps,
                    func=mybir.ActivationFunctionType.Exp,
                    scale=float(H) ** -0.5,
                )
                # causal mask on both diag blocks in one op: keep j >= p
                nc.gpsimd.affine_select(
                    out=mexp[:, 0:2, :], in_=mexp[:, 0:2, :],
                    compare_op=mybir.AluOpType.is_ge,
                    fill=0.0, base=0,
                    pattern=[[0, 2], [1, 128]],
                    channel_multiplier=-1,
                )
                st["mexp"] = mexp

            def stage4(b):
                st = state.pop(b)
                mexp, vsb = st["mexp"], st["vsb"]
                pj = b & 1
                if pj == 0:
                    _PAIR["oa"] = ps_oa.tile([128, 2, TCH, H + 1], fp32,
                                             name=f"oa_{b}", tag="oa")
                oa = _PAIR["oa"]
                nc.tensor.matmul(
                    oa[:, pj, 0, :], lhsT=mexp[:, 0, :], rhs=vsb[:, 0, :],
                    start=True, stop=True,
                )
                nc.tensor.matmul(
                    oa[:, pj, 1, :], lhsT=mexp[:, 2, :], rhs=vsb[:, 0, :],
                    start=True, stop=False,
                )
                nc.tensor.matmul(
                    oa[:, pj, 1, :], lhsT=mexp[:, 1, :], rhs=vsb[:, 1, :],
                    start=False, stop=True,
                )

                if sg["left"] == 0:
                    sg["n"] = next(sg_iter)
                    sg["left"] = sg["n"]
                    sg["start"] = b
                    sg["osb"] = outp.tile([128, sg["n"], TCH, H], fp32,
                                          name=f"osb_{b}",
                                          tag=f"osb{sg['n']}")
                osb = sg["osb"]

                if pj == 1:
                    rec = recp.tile([128, 2, TCH], fp32, tag="rec")
                    nc.vector.reciprocal(rec, oa[:, :, :, H])
                    j = b - 1 - sg["start"]
                    o_ap, r_ap = broadcast_tensor_aps(
                        oa[:, :, :, 0:H], rec[:, :, :, None])
                    nc.vector.tensor_mul(osb[:, j:j + 2], o_ap, r_ap)

                sg["left"] -= 1
                if sg["left"] == 0:
                    n = sg["n"]
                    nc.sync.dma_start(
                        out=out_d[sg["start"]:sg["start"] + n].rearrange(
                            "n (c p) h -> p n c h", p=128),
                        in_=osb,
                    )

            _PAIR = {}
            for i in range(BPC + 3):
                if i < BPC:
                    stage1(i)
                if 1 <= i < BPC + 1:
                    stage2(i - 1)
                if 2 <= i < BPC + 2:
                    stage3(i - 2)
                if 3 <= i:
                    stage4(i - 3)
    nc.compile()
    return nc


def _get_nc():
    if "nc" not in _CACHE:
        _CACHE["nc"] = _build()
    return _CACHE["nc"]


def kernel(x, Wq, Wk, Wv):
    from concourse.bass_utils import run_bass_kernel_spmd

    x = np.ascontiguousarray(np.asarray(x, dtype=np.float32))
    Wq = np.ascontiguousarray(np.asarray(Wq, dtype=np.float32))
    Wk = np.ascontiguousarray(np.asarray(Wk, dtype=np.float32))
    Wv = np.ascontiguousarray(np.asarray(Wv, dtype=np.float32))

    nc = _get_nc()
    in_maps = [
        {"x": x[i * BPC:(i + 1) * BPC], "wq": Wq, "wk": Wk, "wv": Wv}
        for i in range(NCORES)
    ]
    res = run_bass_kernel_spmd(nc, in_maps, list(range(NCORES)))
    return np.concatenate([res.results[i]["out"] for i in range(NCORES)], axis=0)


# revision 8
# speedup vs baseline: 1.0467x; 1.0139x over previous
"""Single-head causal attention kernel for TRN2 (8 NeuronCores, data-parallel).

Problem: x[256,256,384] f32, Wq/Wk/Wv[384,64] f32 ->
  out = softmax(mask((x@Wq)(x@Wk)^T/8)) @ (x@Wv)  [256,256,64] f32

Sharding: batch 256 -> 8 cores x 32 batches. Weights replicated.

v2 dataflow per batch (bf16 matmuls, fp32 PSUM):
  S1: PE transposes x -> xT (6x 128x128), DVE copies xT to SBUF (2x_1p)
  S2: PE qkT = [Wq|Wk]-stationary MM streaming xT (3 MMs, full array)
      -> PSUM [128,2,193]-tile also collects v via 6 xt-stationary MMs;
      ACT copies qT (aligned) + v, DVE copies kT (partition shift 64->0)
  S3: PE scores with kT stationary / qT moving -> wei [diag0|diag1|offdiag];
      ACT exp(scale*wei) -> mexp bf16; Pool one in-place affine_select masks
      both diag blocks
  S4: PE att@v (3 MMs, 65 cols: v plus a persistent ones column for the
      softmax denominator); per PAIR of batches: DVE reciprocal + broadcast
      normalize into the store tile; HWDGE stores groups of 4

PE warmup: dummy transposes fill the initial x-load latency so the tensor
engine p-state is ramped when real work arrives.
"""

import numpy as np

B, T, C, H = 256, 256, 384, 64
NCORES = 8
BPC = B // NCORES  # 32 batches per core
CCH = C // 128  # 3 contraction chunks
TCH = T // 128  # 2 t-chunks

LOADG = [2, 2, 4, 8, 8, 8]  # batches per x-load DMA
STOREG = [4] * 7 + [2, 2]  # batches per out-store DMA (pair-aligned)
NWARM = 20  # dummy PE transposes to ramp the p-state during load latency

_CACHE = {}


def _build():
    import concourse.bass as bass
    import concourse.mybir as mybir
    import concourse.tile as tile
    from concourse import bacc
    from concourse.bass import broadcast_tensor_aps
    from concourse.masks import make_identity

    fp32 = mybir.dt.float32
    bf16 = mybir.dt.bfloat16

    nc = bacc.Bacc()
    x_d = nc.declare_dram_parameter("x", [BPC, T, C], fp32, isOutput=False)
    wq_d = nc.declare_dram_parameter("wq", [C, H], fp32, isOutput=False)
    wk_d = nc.declare_dram_parameter("wk", [C, H], fp32, isOutput=False)
    wv_d = nc.declare_dram_parameter("wv", [C, H], fp32, isOutput=False)
    out_d = nc.declare_dram_parameter("out", [BPC, T, H], fp32, isOutput=True)

    with tile.TileContext(nc) as tc:
        with (
            tc.tile_pool(name="singles", bufs=1) as singles,
            tc.tile_pool(name="xin", bufs=2) as xin,
            tc.tile_pool(name="xtp", bufs=2) as xtp,
            tc.tile_pool(name="qkp", bufs=2) as qkp,
            tc.tile_pool(name="mxp", bufs=2) as mxp,
            tc.tile_pool(name="recp", bufs=2) as recp,
            tc.tile_pool(name="outp", bufs=2) as outp,
            tc.tile_pool(name="ps_xt", bufs=2, space="PSUM") as ps_xt,
            tc.tile_pool(name="ps_qv", bufs=2, space="PSUM") as ps_qv,
            tc.tile_pool(name="ps_wei", bufs=2, space="PSUM") as ps_wei,
            tc.tile_pool(name="ps_oa", bufs=2, space="PSUM") as ps_oa,
        ):
            xbs = []  # (tile, start_batch, nbatch)
            starts = np.cumsum([0] + LOADG)

            def issue_load(gi):
                n = LOADG[gi]
                s = int(starts[gi])
                xb = xin.tile([128, n, TCH, C], bf16, name=f"xb_{gi}",
                              tag=f"xb{n}", bufs=(3 if n == 8 else 2))
                nc.gpsimd.dma_start(
                    out=xb,
                    in_=x_d[s:s + n].rearrange("n (c p) f -> p n c f", p=128),
                )
                xbs.append((xb, s, n))

            # setup order matters: identity first (tiny Pool ops), then the
            # first x-load trigger, then PE warmup can start immediately
            # while W loads go out on the Sync HWDGE queue in parallel.
            ident = singles.tile([128, 128], bf16)
            make_identity(nc, ident)
            issue_load(0)

            # W staging: HWDGE fp32 loads, cast to bf16 on ACT
            # wstage cols 0:64=Wq, 64:128=Wk, 128:192=Wv
            wstage = singles.tile([128, CCH, 3 * H], fp32, tag="wstage")
            for wi, wd in enumerate((wq_d, wk_d, wv_d)):
                nc.sync.dma_start(
                    out=wstage[:, :, wi * H:(wi + 1) * H],
                    in_=wd.rearrange("(c p) h -> p c h", p=128),
                )

            # PE warmup: dummy transposes ramp the p-state during load waits
            for wu in range(NWARM):
                wps = ps_xt.tile([128, CCH, 256], bf16, tag="xt")
                nc.tensor.transpose(wps[:, 0, 0:128], ident, ident)

            for gi in range(1, len(LOADG)):
                issue_load(gi)

            wqk_sb = singles.tile([128, CCH, 2 * H], bf16)
            nc.scalar.copy(wqk_sb, wstage[:, :, 0:2 * H])
            wv_sb = singles.tile([128, CCH, H], bf16)
            nc.scalar.copy(wv_sb, wstage[:, :, 2 * H:3 * H])

            # persistent v tiles with a ones column at [:, :, H]
            vsbs = []
            for vi in range(3):
                vt = singles.tile([128, TCH, H + 1], bf16, name=f"vsb{vi}")
                nc.gpsimd.memset(vt[:, :, H:H + 1], 1.0)
                vsbs.append(vt)

            # --- pipeline state ---
            state = {}  # b -> dict
            ld = {"cur": 0}

            sg_iter = iter(STOREG)
            sg = {"n": 0, "left": 0, "start": 0, "osb": None}

            def stage1(b):
                if b >= xbs[ld["cur"]][1] + xbs[ld["cur"]][2]:
                    ld["cur"] += 1
                xb, xs, xn = xbs[ld["cur"]]
                bi = b - xs

                xt_ps = ps_xt.tile([128, CCH, 256], bf16, tag="xt")
                for c in range(CCH):
                    for t in range(TCH):
                        nc.tensor.transpose(
                            xt_ps[:, c, t * 128:(t + 1) * 128],
                            xb[:, bi, t, c * 128:(c + 1) * 128],
                            ident,
                        )
                xt = xtp.tile([128, CCH, 256], bf16, tag="xt_sb")
                nc.vector.tensor_copy(xt, xt_ps)
                state[b] = {"xt": xt}

            def stage2(b):
                st = state[b]
                xt = st["xt"]
                # qv_ps cols 0:256 = qkT (q rows 0:64, k rows 64:128),
                # cols 256:384 = v as (t h)
                qv_ps = ps_qv.tile([128, 386], fp32)
                for c in range(CCH):
                    nc.tensor.matmul(
                        qv_ps[:, 0:256],
                        lhsT=wqk_sb[:, c, :],
                        rhs=xt[:, c, :],
                        start=(c == 0), stop=(c == CCH - 1),
                    )
                for t in range(TCH):
                    for c in range(CCH):
                        nc.tensor.matmul(
                            qv_ps[:, 256 + t * H:256 + (t + 1) * H],
                            lhsT=xt[:, c, t * 128:(t + 1) * 128],
                            rhs=wv_sb[:, c, :],
                            start=(c == 0), stop=(c == CCH - 1),
                        )
                # qkt[64, qk, 256]: q aligned (ACT), k shifted 64->0 (DVE)
                qkt = qkp.tile([64, 2, 256], bf16, tag="qkt")
                nc.scalar.copy(qkt[:, 0, :], qv_ps[0:64, 0:256])
                nc.vector.tensor_copy(qkt[:, 1, :], qv_ps[64:128, 0:256])
                vsb = vsbs[b % 3]
                nc.scalar.copy(
                    vsb[:, :, 0:H],
                    qv_ps[:, 256:384].rearrange("p (t h) -> p t h", t=TCH))
                st["qkt"] = qkt
                st["vsb"] = vsb

            def stage3(b):
                st = state[b]
                qkt = st["qkt"]
                # wei layout [128, 3, 128]: blk0 = s0 x t0 (diag), blk1 =
                # s1 x t1 (diag), blk2 = s0 x t1 (full)
                wei_ps = ps_wei.tile([128, 3, 128], fp32)
                nc.tensor.matmul(
                    wei_ps[:, 0, :],
                    lhsT=qkt[:, 1, 0:128], rhs=qkt[:, 0, 0:128],
                    start=True, stop=True,
                )
                nc.tensor.matmul(
                    wei_ps[:, 2, :],
                    lhsT=qkt[:, 1, 0:128], rhs=qkt[:, 0, 128:256],
                    start=True, stop=True,
                )
                nc.tensor.matmul(
                    wei_ps[:, 1, :],
                    lhsT=qkt[:, 1, 128:256], rhs=qkt[:, 0, 128:256],
                    start=True, stop=True,
                )
                mexp = mxp.tile([128, 3, 128], bf16, tag="mexp")
                nc.scalar.activation(
                    out=mexp, in_=wei_ps,
                    func=mybir.ActivationFunctionType.Exp,
                    scale=float(H) ** -0.5,
                )
                # causal mask on both diag blocks in one op: keep j >= p
                nc.gpsimd.affine_select(
                    out=mexp[:, 0:2, :], in_=mexp[:, 0:2, :],
                    compare_op=mybir.AluOpType.is_ge,
                    fill=0.0, base=0,
                    pattern=[[0, 2], [1, 128]],
                    channel_multiplier=-1,
                )
                st["mexp"] = mexp

            def stage4(b):
                st = state.pop(b)
                mexp, vsb = st["mexp"], st["vsb"]
                pj = b & 1
                if pj == 0:
                    _PAIR["oa"] = ps_oa.tile([128, 2, TCH, H + 1], fp32,
                                             name=f"oa_{b}", tag="oa")
                oa = _PAIR["oa"]
                nc.tensor.matmul(
                    oa[:, pj, 0, :], lhsT=mexp[:, 0, :], rhs=vsb[:, 0, :],
                    start=True, stop=True,
                )
                nc.tensor.matmul(
                    oa[:, pj, 1, :], lhsT=mexp[:, 2, :], rhs=vsb[:, 0, :],
                    start=True, stop=False,
                )
                nc.tensor.matmul(
                    oa[:, pj, 1, :], lhsT=mexp[:, 1, :], rhs=vsb[:, 1, :],
                    start=False, stop=True,
                )

                if sg["left"] == 0:
                    sg["n"] = next(sg_iter)
                    sg["left"] = sg["n"]
                    sg["start"] = b
                    sg["osb"] = outp.tile([128, sg["n"], TCH, H], fp32,
                                          name=f"osb_{b}",
                                          tag=f"osb{sg['n']}")
                osb = sg["osb"]

                if pj == 1:
                    rec = recp.tile([128, 2, TCH], fp32, tag="rec")
                    nc.vector.reciprocal(rec, oa[:, :, :, H])
                    j = b - 1 - sg["start"]
                    o_ap, r_ap = broadcast_tensor_aps(
                        oa[:, :, :, 0:H], rec[:, :, :, None])
                    nc.vector.tensor_mul(osb[:, j:j + 2], o_ap, r_ap)

                sg["left"] -= 1
                if sg["left"] == 0:
                    n = sg["n"]
                    nc.sync.dma_start(
                        out=out_d[sg["start"]:sg["start"] + n].rearrange(
                            "n (c p) h -> p n c h", p=128),
                        in_=osb,
                    )

            _PAIR = {}
            for i in range(BPC + 3):
                if i < BPC:
                    stage1(i)
                if 1 <= i < BPC + 1:
                    stage2(i - 1)
                if 2 <= i < BPC + 2:
                    stage3(i - 2)
                if 3 <= i:
                    stage4(i - 3)
    nc.compile()
    return nc


def _get_nc():
    if "nc" not in _CACHE:
        _CACHE["nc"] = _build()
    return _CACHE["nc"]


def kernel(x, Wq, Wk, Wv):
    from concourse.bass_utils import run_bass_kernel_spmd

    x = np.ascontiguousarray(np.asarray(x, dtype=np.float32))
    Wq = np.ascontiguousarray(np.asarray(Wq, dtype=np.float32))
    Wk = np.ascontiguousarray(np.asarray(Wk, dtype=np.float32))
    Wv = np.ascontiguousarray(np.asarray(Wv, dtype=np.float32))

    nc = _get_nc()
    in_maps = [
        {"x": x[i * BPC:(i + 1) * BPC], "wq": Wq, "wk": Wk, "wv": Wv}
        for i in range(NCORES)
    ]
    res = run_bass_kernel_spmd(nc, in_maps, list(range(NCORES)))
    return np.concatenate([res.results[i]["out"] for i in range(NCORES)], axis=0)
